# revision 1
# baseline (speedup 1.0000x reference)
"""Trainium2 Bass kernel for nn_BodyAvgDiseaseFeatureAttn2.

Computation (reference):
    attn  = softmax over channels of [heart(27); lung(28); lung(28)] -> [83, 16]
    Weff[o,c,h,w] = attn[o,c] * Wfc[o,c,h,w]
    out[b,o] = mean_s( sum_{c,h,w} x[b,s,c,h,w] * Weff[o,c,h,w] ) + bias[o]

Kernel strategy (pure data parallel, 8 cores, batch-sharded):
  per core (512 volumes):
    - replicate tiny weights; compute softmax/15 and Weff on-device
    - per 128-volume tile: DMA x [128, 8640]; DVE slice-sum -> [128, 576];
      PE-transpose -> [576, 128]; PE matmul with Weff^T -> psum [83, 128];
      bias add -> out sbuf [83, 512]
    - one DMA of [83, 512] out; host assembles [4096, 83]
"""

import numpy as np
from contextlib import ExitStack

import concourse.bass as bass
import concourse.bacc as bacc
import concourse.tile as tile
import concourse.mybir as mybir
from concourse import masks
from concourse.bass_utils import run_bass_kernel_spmd

F32 = mybir.dt.float32
AX = mybir.AxisListType
OP = mybir.AluOpType
ACT = mybir.ActivationFunctionType

N_CORES = 8
B, S, C, H, W = 4096, 15, 16, 6, 6
CK = C * H * W            # 576
SCK = S * CK              # 8640
NH, NL = 27, 28
O = 2 * NL + NH           # 83
BS = B // N_CORES         # 512 volumes per core
P = 128                   # partition tile
NT = BS // P              # 4 batch tiles per core
SH = 8                    # slices in first DMA chunk (second gets 7)
KC = [128, 128, 128, 128, 64]  # contraction chunking of 576


def _build_body(ctx, tc, o_d, x_d, h_d, l_d, w_d, b_d):
    nc = tc.nc

    const = ctx.enter_context(tc.tile_pool(name="const", bufs=1))
    ptr = ctx.enter_context(tc.tile_pool(name="ptr", bufs=4, space="PSUM"))
    pout = ctx.enter_context(tc.tile_pool(name="pout", bufs=2, space="PSUM"))
    xpool = ctx.enter_context(tc.tile_pool(name="xp", bufs=12))
    spool = ctx.enter_context(tc.tile_pool(name="sp", bufs=8))
    hpool = ctx.enter_context(tc.tile_pool(name="hp", bufs=3))
    xtp = ctx.enter_context(tc.tile_pool(name="xtp", bufs=2))

    # ---- constants / setup --------------------------------------------
    ident = const.tile([128, 128], F32)
    masks.make_identity(nc, ident[:])

    attn = const.tile([O, 16], F32)
    nc.scalar.dma_start(attn[0:NH, :], h_d[:, :])
    nc.scalar.dma_start(attn[NH:NH + NL, :], l_d[:, :])
    nc.scalar.dma_start(attn[NH + NL:O, :], l_d[:, :])

    wsb = const.tile([O, CK], F32)
    nc.scalar.dma_start(wsb[:], w_d[:, :])
    bias = const.tile([O, 1], F32)
    nc.scalar.dma_start(bias[:], b_d[:, :])

    # softmax over the 16 channels, folded with the 1/S slice-average
    negmax = const.tile([O, 1], F32)
    nc.vector.tensor_reduce(negmax[:], attn[:], axis=AX.X, op=OP.max, negate=True)
    att_e = const.tile([O, 16], F32)
    den = const.tile([O, 1], F32)
    nc.scalar.activation(att_e[:], attn[:], ACT.Exp, bias=negmax[:, :], scale=1.0,
                         accum_out=den[:])
    den_s = const.tile([O, 1], F32)
    nc.scalar.mul(den_s[:], den[:], float(S))
    rden = const.tile([O, 1], F32)
    nc.vector.reciprocal(rden[:], den_s[:])
    attn_n = const.tile([O, 16], F32)
    nc.vector.tensor_scalar_mul(attn_n[:], att_e[:], rden[:, :])

    # Weff[o, c, k] = attn_n[o, c] * Wfc[o, c, k]   (k = 36 spatial)
    weff = const.tile([O, CK], F32)
    w_v = wsb[:].rearrange("p (c k) -> p c k", c=C)
    a_v = attn_n[:].rearrange("p (c k) -> p c k", k=1)
    o_v = weff[:].rearrange("p (c k) -> p c k", c=C)
    w_bc, a_bc = bass.broadcast_tensor_aps(w_v, a_v)
    nc.vector.tensor_tensor(o_v, w_bc, a_bc, op=OP.mult)

    # Weff^T chunks: wT[:, k*O:(k+1)*O] holds Weff[:, k-chunk].T  ([kw, 83])
    wT = const.tile([128, 5 * O], F32)
    for k, kw in enumerate(KC):
        c0 = 128 * k
        pt = ptr.tile([128, 128], F32, tag="pt")
        nc.tensor.transpose(pt[0:kw, 0:O], weff[:, c0:c0 + kw], ident[0:O, 0:O])
        nc.scalar.copy(wT[0:kw, k * O:(k + 1) * O], pt[0:kw, 0:O])

    outsb = const.tile([O, BS], F32)

    # ---- main loop over batch tiles -----------------------------------
    # x tile is loaded in 4 slice-aligned chunks: 4+4+4+3 slices.
    QS = [(0, 4), (4, 4), (8, 4), (12, 3)]
    for t in range(NT):
        b0 = t * P
        qs = []
        for (s0, ns) in QS:
            xq = xpool.tile([P, 4 * CK], F32, tag="xq")
            nc.sync.dma_start(xq[:, 0:ns * CK],
                             x_d[b0:b0 + P, s0 * CK:(s0 + ns) * CK])
            qs.append(xq)

        xT = xtp.tile([128, 5 * P], F32)
        if t < NT - 1:
            # contiguous pairwise tree slice-sum, split DVE / GPSIMD:
            #   DVE:  h0 h1 r0 r1 u      GPSIMD: h2 h3 r2 r3 v      DVE: ss
            h0 = hpool.tile([P, 2 * CK], F32, tag="h")
            nc.vector.tensor_add(h0[:], qs[0][:, 0:2 * CK], qs[0][:, 2 * CK:4 * CK])
            h1 = hpool.tile([P, 2 * CK], F32, tag="h")
            nc.vector.tensor_add(h1[:], qs[1][:, 0:2 * CK], qs[1][:, 2 * CK:4 * CK])
            h2 = hpool.tile([P, 2 * CK], F32, tag="h")
            nc.gpsimd.tensor_add(h2[:], qs[2][:, 0:2 * CK], qs[2][:, 2 * CK:4 * CK])
            h3 = spool.tile([P, CK], F32, tag="s")
            nc.gpsimd.tensor_add(h3[:], qs[3][:, 0:CK], qs[3][:, CK:2 * CK])
            r0 = spool.tile([P, CK], F32, tag="s")
            nc.vector.tensor_add(r0[:], h0[:, 0:CK], h0[:, CK:2 * CK])
            r1 = spool.tile([P, CK], F32, tag="s")
            nc.vector.tensor_add(r1[:], h1[:, 0:CK], h1[:, CK:2 * CK])
            r2 = spool.tile([P, CK], F32, tag="s")
            nc.gpsimd.tensor_add(r2[:], h2[:, 0:CK], h2[:, CK:2 * CK])
            r3 = spool.tile([P, CK], F32, tag="s")
            nc.gpsimd.tensor_add(r3[:], h3[:], qs[3][:, 2 * CK:3 * CK])
            u = spool.tile([P, CK], F32, tag="s")
            nc.vector.tensor_add(u[:], r0[:], r1[:])
            v = spool.tile([P, CK], F32, tag="s")
            nc.gpsimd.tensor_add(v[:], r2[:], r3[:])
            ss = spool.tile([P, CK], F32, tag="ss")
            nc.vector.tensor_add(ss[:], u[:], v[:])
            # PE transpose the slice-sum: xT[:, k*P:(k+1)*P] = ss[:, chunk].T
            for k, kw in enumerate(KC):
                c0 = 128 * k
                pt = ptr.tile([128, 128], F32, tag="pt")
                nc.tensor.transpose(pt[0:kw, :], ss[:, c0:c0 + kw], ident[:, :])
                if k % 2 == 0:
                    nc.scalar.copy(xT[0:kw, k * P:(k + 1) * P], pt[0:kw, :])
                else:
                    nc.vector.tensor_copy(xT[0:kw, k * P:(k + 1) * P], pt[0:kw, :])
        else:
            # Last tile: short-tail form. Only one level of adds; the rest of
            # the slice-sum accumulates in PSUM across 8 PE transposes, with
            # the q3-dependent blocks ordered LAST so the post-DMA critical
            # chain is one DVE add + the tail of each transpose group.
            h0 = hpool.tile([P, 2 * CK], F32, tag="h")
            nc.vector.tensor_add(h0[:], qs[0][:, 0:2 * CK], qs[0][:, 2 * CK:4 * CK])
            h1 = hpool.tile([P, 2 * CK], F32, tag="h")
            nc.vector.tensor_add(h1[:], qs[1][:, 0:2 * CK], qs[1][:, 2 * CK:4 * CK])
            h2 = hpool.tile([P, 2 * CK], F32, tag="h")
            nc.gpsimd.tensor_add(h2[:], qs[2][:, 0:2 * CK], qs[2][:, 2 * CK:4 * CK])
            h3 = spool.tile([P, CK], F32, tag="s")
            nc.vector.tensor_add(h3[:], qs[3][:, 0:CK], qs[3][:, CK:2 * CK])
            blocks = [(h0, 0), (h0, CK), (h1, 0), (h1, CK), (h2, 0), (h2, CK),
                      (h3, 0), (qs[3], 2 * CK)]
            for k, kw in enumerate(KC):
                c0 = 128 * k
                pt = ptr.tile([128, 128], F32, tag="pt")
                for j, (blk, off) in enumerate(blocks):
                    nc.tensor.matmul(pt[0:kw, :], blk[:, off + c0:off + c0 + kw],
                                     ident[:, :], is_transpose=True,
                                     start=(j == 0), stop=(j == len(blocks) - 1))
                if k % 2 == 0:
                    nc.scalar.copy(xT[0:kw, k * P:(k + 1) * P], pt[0:kw, :])
                else:
                    nc.vector.tensor_copy(xT[0:kw, k * P:(k + 1) * P], pt[0:kw, :])

        po = pout.tile([O, P], F32)
        for k, kw in enumerate(KC):
            nc.tensor.matmul(po[:], wT[0:kw, k * O:(k + 1) * O],
                             xT[0:kw, k * P:(k + 1) * P],
                             start=(k == 0), stop=(k == len(KC) - 1))

        nc.vector.tensor_scalar_add(outsb[:, b0:b0 + P], po[:], bias[:, :])

    nc.sync.dma_start(o_d[:, :], outsb[:])


def build_program(repeat: int = 1):
    nc = bacc.Bacc("TRN2", target_bir_lowering=False, debug=False,
                   num_devices=N_CORES)
    x_d = nc.dram_tensor("x", [BS, SCK], F32, kind="ExternalInput").ap()
    h_d = nc.dram_tensor("heart", [NH, 16], F32, kind="ExternalInput").ap()
    l_d = nc.dram_tensor("lung", [NL, 16], F32, kind="ExternalInput").ap()
    w_d = nc.dram_tensor("fcw", [O, CK], F32, kind="ExternalInput").ap()
    b_d = nc.dram_tensor("fcb", [O, 1], F32, kind="ExternalInput").ap()
    o_d = nc.dram_tensor("out", [O, BS], F32, kind="ExternalOutput").ap()

    with tile.TileContext(nc) as tc:
        if repeat == 1:
            with ExitStack() as ctx:
                _build_body(ctx, tc, o_d, x_d, h_d, l_d, w_d, b_d)
        else:
            def body(_iv):
                with ExitStack() as ctx:
                    _build_body(ctx, tc, o_d, x_d, h_d, l_d, w_d, b_d)
            tc.For_i_unrolled(0, repeat, 1, body, max_unroll=1)
    nc.compile()
    return nc


_NC_CACHE = {}


def _get_program(repeat: int = 1):
    if repeat not in _NC_CACHE:
        _NC_CACHE[repeat] = build_program(repeat)
    return _NC_CACHE[repeat]


class _Runner:
    """Cached jitted shard_map runner (mirrors bass2jax.run_bass_via_pjrt's
    multi-core path, but built once and fed full arrays without the per-core
    split + re-concat host copies)."""

    def __init__(self, nc):
        import jax
        from jax.sharding import Mesh, PartitionSpec, NamedSharding
        from jax.experimental.shard_map import shard_map
        from concourse import bass2jax
        from concourse.bass2jax import _bass_exec_p, install_neuronx_cc_hook

        install_neuronx_cc_hook()
        self.jax = jax
        pname = nc.partition_id_tensor.name if nc.partition_id_tensor else None
        in_names, out_names, out_avals, zeros = [], [], [], []
        for alloc in nc.m.functions[0].allocations:
            if not isinstance(alloc, mybir.MemoryLocationSet):
                continue
            name = alloc.memorylocations[0].name
            if alloc.kind == "ExternalInput":
                if name != pname:
                    in_names.append(name)
            elif alloc.kind == "ExternalOutput":
                shape = tuple(alloc.tensor_shape)
                dtype = mybir.dt.np(alloc.dtype)
                out_names.append(name)
                out_avals.append(jax.core.ShapedArray(shape, dtype))
                zeros.append(np.zeros((N_CORES * shape[0], *shape[1:]), dtype))
        self.in_names, self.out_names, self.zeros = in_names, out_names, zeros
        all_in = list(in_names) + list(out_names)
        if pname is not None:
            all_in.append(pname)

        def _body(*args):
            operands = list(args)
            if pname is not None:
                operands.append(bass2jax.partition_id_tensor())
            return tuple(_bass_exec_p.bind(
                *operands, out_avals=tuple(out_avals), in_names=tuple(all_in),
                out_names=tuple(out_names), lowering_input_output_aliases=(),
                sim_require_finite=True, sim_require_nnan=True, nc=nc))

        devices = jax.devices()[:N_CORES]
        mesh = Mesh(np.asarray(devices), ("core",))
        n_p, n_o = len(in_names), len(out_names)
        self.sharded = jax.jit(
            shard_map(_body, mesh=mesh,
                      in_specs=(PartitionSpec("core"),) * (n_p + n_o),
                      out_specs=(PartitionSpec("core"),) * n_o,
                      check_rep=False),
            donate_argnums=tuple(range(n_p, n_p + n_o)), keep_unused=True)
        self.sharding = NamedSharding(mesh, PartitionSpec("core"))

    def __call__(self, full_ins: dict):
        outs = self.sharded(*[full_ins[n] for n in self.in_names],
                            *[z.copy() for z in self.zeros])
        return {n: np.asarray(outs[i]) for i, n in enumerate(self.out_names)}


_RUNNER = None


def make_in_maps(inputs):
    x = np.asarray(inputs["x"], dtype=np.float32).reshape(B, SCK)
    h = np.asarray(inputs["dzfeatweights_heart"], dtype=np.float32).reshape(NH, 16)
    l = np.asarray(inputs["dzfeatweights_lung"], dtype=np.float32).reshape(NL, 16)
    w = np.asarray(inputs["fclayers_weights"], dtype=np.float32).reshape(O, CK)
    b = np.asarray(inputs["fclayers_biases"], dtype=np.float32).reshape(O, 1)
    return [{"x": x[c * BS:(c + 1) * BS], "heart": h, "lung": l, "fcw": w, "fcb": b}
            for c in range(N_CORES)]


def assemble_output(results):
    outs = [results[c]["out"] for c in range(N_CORES)]    # each [83, 512]
    return np.ascontiguousarray(np.concatenate(outs, axis=1).T)  # [4096, 83]


def kernel(**inputs) -> np.ndarray:
    global _RUNNER
    if _RUNNER is None:
        _RUNNER = _Runner(_get_program(1))
    # Full (concatenated-over-cores) input arrays; x needs no copy at all.
    full = {
        "x": np.ascontiguousarray(
            np.asarray(inputs["x"], dtype=np.float32)).reshape(B, SCK),
        "heart": np.tile(np.asarray(inputs["dzfeatweights_heart"],
                                    dtype=np.float32).reshape(NH, 16),
                         (N_CORES, 1)),
        "lung": np.tile(np.asarray(inputs["dzfeatweights_lung"],
                                   dtype=np.float32).reshape(NL, 16),
                        (N_CORES, 1)),
        "fcw": np.tile(np.asarray(inputs["fclayers_weights"],
                                  dtype=np.float32).reshape(O, CK),
                       (N_CORES, 1)),
        "fcb": np.tile(np.asarray(inputs["fclayers_biases"],
                                  dtype=np.float32).reshape(O, 1),
                       (N_CORES, 1)),
    }
    outs = _RUNNER(full)["out"]            # [8*83, 512]
    per_core = outs.reshape(N_CORES, O, BS)
    return np.ascontiguousarray(
        np.concatenate([per_core[c] for c in range(N_CORES)], axis=1).T)



# revision 4
# speedup vs baseline: 2.7390x; 2.7390x over previous
"""Trainium2 Bass kernel for nn_BodyAvgDiseaseFeatureAttn2.

Computation (reference):
    attn  = softmax over channels of [heart(27); lung(28); lung(28)] -> [83, 16]
    Weff[o,c,h,w] = attn[o,c] * Wfc[o,c,h,w]
    out[b,o] = mean_s( sum_{c,h,w} x[b,s,c,h,w] * Weff[o,c,h,w] ) + bias[o]

Kernel strategy (pure data parallel, 8 cores, batch-sharded), v2:
  - x per core is 17.7 MB; HBM stream at ~428 GB/s is the roofline
    (~41.5 us). All 4 batch tiles (138 KB/partition) fit in SBUF, so ALL
    x DMAs are issued up front (20 chunk DMAs, no buffer reuse -> no
    stalls), and compute overlaps the stream with large headroom.
  - slice-sum tree: fp32 pair-adds split DVE (chunks 0,1 + combines) /
    GPSIMD (chunk 2 + tail combines), fused via 3D APs (2 pairs/op).
  - final 576-dot on PE in BF16 (tree output downcast in its last add;
    Weff precomputed in bf16): transposes+matmuls ~2x faster, fp32 PSUM
    accumulation keeps error ~1e-3 << 2e-2 budget.
  - psum->sbuf copies and the bias add run on the Scalar engine.
  - output store split into 4 row-chunks on 4 queues (a single [83 x
    2 KB] store serializes ~95 ns/packet on one DMA engine: ~8 us).
"""

import numpy as np
from contextlib import ExitStack

import concourse.bass as bass
import concourse.bacc as bacc
import concourse.tile as tile
import concourse.mybir as mybir
from concourse import masks
from concourse.bass_utils import run_bass_kernel_spmd

F32 = mybir.dt.float32
BF16 = mybir.dt.bfloat16
AX = mybir.AxisListType
OP = mybir.AluOpType
ACT = mybir.ActivationFunctionType

N_CORES = 8
B, S, C, H, W = 4096, 15, 16, 6, 6
CK = C * H * W            # 576
SCK = S * CK              # 8640
NH, NL = 27, 28
O = 2 * NL + NH           # 83
BS = B // N_CORES         # 512 volumes per core
P = 128                   # partition tile
NT = BS // P              # 4 batch tiles per core
KC = [128, 128, 128, 128, 64]  # contraction chunking of 576
# slice chunking of each tile's DMA: 4+4+4+2+1 slices
QS = [(0, 4), (4, 4), (8, 4), (12, 2), (14, 1)]


def _build_body(ctx, tc, o_d, x_d, h_d, l_d, w_d, b_d):
    nc = tc.nc

    const = ctx.enter_context(tc.tile_pool(name="const", bufs=1))
    ptr = ctx.enter_context(tc.tile_pool(name="ptr", bufs=4, space="PSUM"))
    pout = ctx.enter_context(tc.tile_pool(name="pout", bufs=2, space="PSUM"))
    xpool = ctx.enter_context(tc.tile_pool(name="xp", bufs=1))
    apool = ctx.enter_context(tc.tile_pool(name="ap", bufs=1))
    spool = ctx.enter_context(tc.tile_pool(name="sp", bufs=1))
    sspool = ctx.enter_context(tc.tile_pool(name="ssp", bufs=2))
    xtp = ctx.enter_context(tc.tile_pool(name="xtp", bufs=2))

    # ---- x DMAs issued first: all tiles fully prefetched ---------------
    qs = [[] for _ in range(NT)]
    for t in range(NT):
        b0 = t * P
        for (s0, ns) in QS:
            xq = xpool.tile([P, ns * CK], F32, tag=f"xq{t}_{s0}")
            nc.sync.dma_start(xq[:], x_d[b0:b0 + P, s0 * CK:(s0 + ns) * CK])
            qs[t].append(xq)

    # ---- constants / setup ---------------------------------------------
    ident = const.tile([128, 128], BF16)
    masks.make_identity(nc, ident[:])

    attn = const.tile([O, 16], F32)
    nc.scalar.dma_start(attn[0:NH, :], h_d[:, :])
    nc.scalar.dma_start(attn[NH:NH + NL, :], l_d[:, :])
    nc.scalar.dma_start(attn[NH + NL:O, :], l_d[:, :])

    wsb = const.tile([O, CK], F32)
    nc.scalar.dma_start(wsb[:], w_d[:, :])
    bias = const.tile([O, 1], F32)
    nc.scalar.dma_start(bias[:], b_d[:, :])

    # softmax over the 16 channels, folded with the 1/S slice-average
    negmax = const.tile([O, 1], F32)
    nc.vector.tensor_reduce(negmax[:], attn[:], axis=AX.X, op=OP.max, negate=True)
    att_e = const.tile([O, 16], F32)
    den = const.tile([O, 1], F32)
    nc.scalar.activation(att_e[:], attn[:], ACT.Exp, bias=negmax[:, :], scale=1.0,
                         accum_out=den[:])
    den_s = const.tile([O, 1], F32)
    nc.scalar.mul(den_s[:], den[:], float(S))
    rden = const.tile([O, 1], F32)
    nc.vector.reciprocal(rden[:], den_s[:])
    attn_n = const.tile([O, 16], F32)
    nc.vector.tensor_scalar_mul(attn_n[:], att_e[:], rden[:, :])

    # Weff[o, c, k] = attn_n[o, c] * Wfc[o, c, k]   (k = 36 spatial), bf16
    weff = const.tile([O, CK], BF16)
    w_v = wsb[:].rearrange("p (c k) -> p c k", c=C)
    a_v = attn_n[:].rearrange("p (c k) -> p c k", k=1)
    o_v = weff[:].rearrange("p (c k) -> p c k", c=C)
    w_bc, a_bc = bass.broadcast_tensor_aps(w_v, a_v)
    nc.vector.tensor_tensor(o_v, w_bc, a_bc, op=OP.mult)

    # Weff^T chunks (bf16): wT[:, k*O:(k+1)*O] = Weff[:, chunk].T  ([kw, 83])
    wT = const.tile([128, 5 * O], BF16)
    for k, kw in enumerate(KC):
        c0 = 128 * k
        pt = ptr.tile([128, 128], BF16, tag="pt")
        nc.tensor.transpose(pt[0:kw, 0:O], weff[:, c0:c0 + kw], ident[0:O, 0:O])
        nc.scalar.copy(wT[0:kw, k * O:(k + 1) * O], pt[0:kw, 0:O])

    outsb = const.tile([O, BS], F32)

    # ---- main loop over batch tiles ------------------------------------
    for t in range(NT):
        b0 = t * P
        q0, q1, q2, q3, q4 = qs[t]

        # level A: pair adds within 4-slice chunks (2 pairs fused per op)
        def pairs(q):
            v = q[:].rearrange("p (a b k) -> p a b k", a=2, b=2)
            return v[:, :, 0, :], v[:, :, 1, :]

        a0 = apool.tile([P, 2 * CK], F32, tag="a0")
        s0a, s0b = pairs(q0)
        nc.vector.tensor_tensor(a0[:].rearrange("p (a k) -> p a k", a=2),
                                s0a, s0b, op=OP.add)
        a1 = apool.tile([P, 2 * CK], F32, tag="a1")
        s1a, s1b = pairs(q1)
        nc.vector.tensor_tensor(a1[:].rearrange("p (a k) -> p a k", a=2),
                                s1a, s1b, op=OP.add)
        a2 = apool.tile([P, 2 * CK], F32, tag="a2")
        s2a, s2b = pairs(q2)
        nc.gpsimd.tensor_tensor(a2[:].rearrange("p (a k) -> p a k", a=2),
                                s2a, s2b, op=OP.add)
        a3 = spool.tile([P, CK], F32, tag="a3")
        nc.gpsimd.tensor_add(a3[:], q3[:, 0:CK], q3[:, CK:2 * CK])

        # level B
        b0t = spool.tile([P, CK], F32, tag="b0")
        nc.vector.tensor_add(b0t[:], a0[:, 0:CK], a0[:, CK:2 * CK])
        b1t = spool.tile([P, CK], F32, tag="b1")
        nc.vector.tensor_add(b1t[:], a1[:, 0:CK], a1[:, CK:2 * CK])
        b2t = spool.tile([P, CK], F32, tag="b2")
        nc.gpsimd.tensor_add(b2t[:], a2[:, 0:CK], a2[:, CK:2 * CK])

        # level C + D; last add downcasts to bf16 for the PE stage
        c0t = spool.tile([P, CK], F32, tag="c0")
        nc.vector.tensor_add(c0t[:], b0t[:], b1t[:])
        c1t = spool.tile([P, CK], F32, tag="c1")
        nc.gpsimd.tensor_add(c1t[:], b2t[:], a3[:])
        ssa = spool.tile([P, CK], F32, tag="ssa")
        nc.vector.tensor_add(ssa[:], c0t[:], q4[:])
        ss = sspool.tile([P, CK], BF16, tag="ss")
        nc.vector.tensor_add(ss[:], ssa[:], c1t[:])

        # PE transpose the slice-sum: xT[:, k*P:(k+1)*P] = ss[:, chunk].T
        xT = xtp.tile([128, 5 * P], BF16)
        for k, kw in enumerate(KC):
            c0 = 128 * k
            pt = ptr.tile([128, 128], BF16, tag="pt")
            nc.tensor.transpose(pt[0:kw, :], ss[:, c0:c0 + kw], ident[:, :])
            nc.scalar.copy(xT[0:kw, k * P:(k + 1) * P], pt[0:kw, :])

        po = pout.tile([O, P], F32)
        for k, kw in enumerate(KC):
            nc.tensor.matmul(po[:], wT[0:kw, k * O:(k + 1) * O],
                             xT[0:kw, k * P:(k + 1) * P],
                             start=(k == 0), stop=(k == len(KC) - 1))

        nc.scalar.add(outsb[:, b0:b0 + P], po[:], bias[:, :])

    # ---- output store: 3 row-chunks on the 3 DMA-capable queues --------
    RS = [(0, 28), (28, 28), (56, 27)]
    engines = [nc.sync, nc.scalar, nc.gpsimd]
    for (r0, nr), eng in zip(RS, engines):
        eng.dma_start(o_d[r0:r0 + nr, :], outsb[r0:r0 + nr, :])


def build_program(repeat: int = 1):
    nc = bacc.Bacc("TRN2", target_bir_lowering=False, debug=False,
                   num_devices=N_CORES)
    x_d = nc.dram_tensor("x", [BS, SCK], F32, kind="ExternalInput").ap()
    h_d = nc.dram_tensor("heart", [NH, 16], F32, kind="ExternalInput").ap()
    l_d = nc.dram_tensor("lung", [NL, 16], F32, kind="ExternalInput").ap()
    w_d = nc.dram_tensor("fcw", [O, CK], F32, kind="ExternalInput").ap()
    b_d = nc.dram_tensor("fcb", [O, 1], F32, kind="ExternalInput").ap()
    o_d = nc.dram_tensor("out", [O, BS], F32, kind="ExternalOutput").ap()

    with tile.TileContext(nc) as tc:
        if repeat == 1:
            with ExitStack() as ctx:
                _build_body(ctx, tc, o_d, x_d, h_d, l_d, w_d, b_d)
        else:
            def body(_iv):
                with ExitStack() as ctx:
                    _build_body(ctx, tc, o_d, x_d, h_d, l_d, w_d, b_d)
            tc.For_i_unrolled(0, repeat, 1, body, max_unroll=1)
    nc.compile()
    return nc


_NC_CACHE = {}


def _get_program(repeat: int = 1):
    if repeat not in _NC_CACHE:
        _NC_CACHE[repeat] = build_program(repeat)
    return _NC_CACHE[repeat]


class _Runner:
    """Cached jitted shard_map runner (mirrors bass2jax.run_bass_via_pjrt's
    multi-core path, but built once and fed full arrays without the per-core
    split + re-concat host copies)."""

    def __init__(self, nc):
        import jax
        from jax.sharding import Mesh, PartitionSpec, NamedSharding
        from jax.experimental.shard_map import shard_map
        from concourse import bass2jax
        from concourse.bass2jax import _bass_exec_p, install_neuronx_cc_hook

        install_neuronx_cc_hook()
        self.jax = jax
        pname = nc.partition_id_tensor.name if nc.partition_id_tensor else None
        in_names, out_names, out_avals, zeros = [], [], [], []
        for alloc in nc.m.functions[0].allocations:
            if not isinstance(alloc, mybir.MemoryLocationSet):
                continue
            name = alloc.memorylocations[0].name
            if alloc.kind == "ExternalInput":
                if name != pname:
                    in_names.append(name)
            elif alloc.kind == "ExternalOutput":
                shape = tuple(alloc.tensor_shape)
                dtype = mybir.dt.np(alloc.dtype)
                out_names.append(name)
                out_avals.append(jax.core.ShapedArray(shape, dtype))
                zeros.append(np.zeros((N_CORES * shape[0], *shape[1:]), dtype))
        self.in_names, self.out_names, self.zeros = in_names, out_names, zeros
        all_in = list(in_names) + list(out_names)
        if pname is not None:
            all_in.append(pname)

        def _body(*args):
            operands = list(args)
            if pname is not None:
                operands.append(bass2jax.partition_id_tensor())
            return tuple(_bass_exec_p.bind(
                *operands, out_avals=tuple(out_avals), in_names=tuple(all_in),
                out_names=tuple(out_names), lowering_input_output_aliases=(),
                sim_require_finite=True, sim_require_nnan=True, nc=nc))

        devices = jax.devices()[:N_CORES]
        mesh = Mesh(np.asarray(devices), ("core",))
        n_p, n_o = len(in_names), len(out_names)
        self.sharded = jax.jit(
            shard_map(_body, mesh=mesh,
                      in_specs=(PartitionSpec("core"),) * (n_p + n_o),
                      out_specs=(PartitionSpec("core"),) * n_o,
                      check_rep=False),
            donate_argnums=tuple(range(n_p, n_p + n_o)), keep_unused=True)
        self.sharding = NamedSharding(mesh, PartitionSpec("core"))

    def __call__(self, full_ins: dict):
        outs = self.sharded(*[full_ins[n] for n in self.in_names],
                            *[z.copy() for z in self.zeros])
        return {n: np.asarray(outs[i]) for i, n in enumerate(self.out_names)}


_RUNNER = None


def make_in_maps(inputs):
    x = np.asarray(inputs["x"], dtype=np.float32).reshape(B, SCK)
    h = np.asarray(inputs["dzfeatweights_heart"], dtype=np.float32).reshape(NH, 16)
    l = np.asarray(inputs["dzfeatweights_lung"], dtype=np.float32).reshape(NL, 16)
    w = np.asarray(inputs["fclayers_weights"], dtype=np.float32).reshape(O, CK)
    b = np.asarray(inputs["fclayers_biases"], dtype=np.float32).reshape(O, 1)
    return [{"x": x[c * BS:(c + 1) * BS], "heart": h, "lung": l, "fcw": w, "fcb": b}
            for c in range(N_CORES)]


def assemble_output(results):
    outs = [results[c]["out"] for c in range(N_CORES)]    # each [83, 512]
    return np.ascontiguousarray(np.concatenate(outs, axis=1).T)  # [4096, 83]


def kernel(**inputs) -> np.ndarray:
    global _RUNNER
    if _RUNNER is None:
        _RUNNER = _Runner(_get_program(1))
    # Full (concatenated-over-cores) input arrays; x needs no copy at all.
    full = {
        "x": np.ascontiguousarray(
            np.asarray(inputs["x"], dtype=np.float32)).reshape(B, SCK),
        "heart": np.tile(np.asarray(inputs["dzfeatweights_heart"],
                                    dtype=np.float32).reshape(NH, 16),
                         (N_CORES, 1)),
        "lung": np.tile(np.asarray(inputs["dzfeatweights_lung"],
                                   dtype=np.float32).reshape(NL, 16),
                        (N_CORES, 1)),
        "fcw": np.tile(np.asarray(inputs["fclayers_weights"],
                                  dtype=np.float32).reshape(O, CK),
                       (N_CORES, 1)),
        "fcb": np.tile(np.asarray(inputs["fclayers_biases"],
                                  dtype=np.float32).reshape(O, 1),
                       (N_CORES, 1)),
    }
    outs = _RUNNER(full)["out"]            # [8*83, 512]
    per_core = outs.reshape(N_CORES, O, BS)
    return np.ascontiguousarray(
        np.concatenate([per_core[c] for c in range(N_CORES)], axis=1).T)


# revision 7
# speedup vs baseline: 2.8554x; 1.0425x over previous
"""Trainium2 Bass kernel for nn_BodyAvgDiseaseFeatureAttn2.

Computation (reference):
    attn  = softmax over channels of [heart(27); lung(28); lung(28)] -> [83, 16]
    Weff[o,c,h,w] = attn[o,c] * Wfc[o,c,h,w]
    out[b,o] = mean_s( sum_{c,h,w} x[b,s,c,h,w] * Weff[o,c,h,w] ) + bias[o]

Kernel strategy (pure data parallel, 8 cores, batch-sharded), v2:
  - x per core is 17.7 MB; HBM stream at ~428 GB/s is the roofline
    (~41.5 us). All 4 batch tiles (138 KB/partition) fit in SBUF, so ALL
    x DMAs are issued up front (20 chunk DMAs, no buffer reuse -> no
    stalls), and compute overlaps the stream with large headroom.
  - slice-sum tree: fp32 pair-adds split DVE (chunks 0,1 + combines) /
    GPSIMD (chunk 2 + tail combines), fused via 3D APs (2 pairs/op).
  - final 576-dot on PE in BF16 (tree output downcast in its last add;
    Weff precomputed in bf16): transposes+matmuls ~2x faster, fp32 PSUM
    accumulation keeps error ~1e-3 << 2e-2 budget.
  - psum->sbuf copies and the bias add run on the Scalar engine.
  - output store split into 4 row-chunks on 4 queues (a single [83 x
    2 KB] store serializes ~95 ns/packet on one DMA engine: ~8 us).
"""

import numpy as np
from contextlib import ExitStack

import concourse.bass as bass
import concourse.bacc as bacc
import concourse.tile as tile
import concourse.mybir as mybir
from concourse import masks
from concourse.bass_utils import run_bass_kernel_spmd

F32 = mybir.dt.float32
BF16 = mybir.dt.bfloat16
AX = mybir.AxisListType
OP = mybir.AluOpType
ACT = mybir.ActivationFunctionType

N_CORES = 8
B, S, C, H, W = 4096, 15, 16, 6, 6
CK = C * H * W            # 576
SCK = S * CK              # 8640
NH, NL = 27, 28
O = 2 * NL + NH           # 83
BS = B // N_CORES         # 512 volumes per core
P = 128                   # partition tile
NT = BS // P              # 4 batch tiles per core
KC = [128, 128, 128, 128, 64]  # contraction chunking of 576
# slice chunking of each tile's DMA: 4+4+4+2+1 slices
QS = [(0, 4), (4, 4), (8, 4), (12, 2), (14, 1)]


def _build_body(ctx, tc, o_d, x_d, s_d):
    nc = tc.nc

    const = ctx.enter_context(tc.tile_pool(name="const", bufs=1))
    ptr = ctx.enter_context(tc.tile_pool(name="ptr", bufs=4, space="PSUM"))
    pout = ctx.enter_context(tc.tile_pool(name="pout", bufs=2, space="PSUM"))
    xpool = ctx.enter_context(tc.tile_pool(name="xp", bufs=1))
    apool = ctx.enter_context(tc.tile_pool(name="ap", bufs=1))
    spool = ctx.enter_context(tc.tile_pool(name="sp", bufs=1))
    sspool = ctx.enter_context(tc.tile_pool(name="ssp", bufs=2))
    xtp = ctx.enter_context(tc.tile_pool(name="xtp", bufs=2))

    # ---- setup DMA (one packed [83, 593] tensor), then all x DMAs ------
    # All on the sync queue: 83 + 20*128 read packets spread across the 16
    # DMA engines. Separate small DMAs on the scalar queue serialize at
    # ~95 ns/packet on one engine and (via completion-semaphore reuse)
    # stall the x stream.
    setup = const.tile([O, 16 + CK + 1], F32)
    nc.sync.dma_start(setup[:], s_d[:, :])
    attn = setup[:, 0:16]
    wsb = setup[:, 16:16 + CK]
    bias = setup[:, 16 + CK:16 + CK + 1]

    qs = [[] for _ in range(NT)]
    for t in range(NT):
        b0 = t * P
        for (s0, ns) in QS:
            xq = xpool.tile([P, ns * CK], F32, tag=f"xq{t}_{s0}")
            nc.sync.dma_start(xq[:], x_d[b0:b0 + P, s0 * CK:(s0 + ns) * CK])
            qs[t].append(xq)

    # ---- constants / setup ---------------------------------------------
    ident = const.tile([128, 128], BF16)
    masks.make_identity(nc, ident[:])

    # softmax over the 16 channels, folded with the 1/S slice-average
    negmax = const.tile([O, 1], F32)
    nc.vector.tensor_reduce(negmax[:], attn, axis=AX.X, op=OP.max, negate=True)
    att_e = const.tile([O, 16], F32)
    den = const.tile([O, 1], F32)
    nc.scalar.activation(att_e[:], attn, ACT.Exp, bias=negmax[:, :], scale=1.0,
                         accum_out=den[:])
    den_s = const.tile([O, 1], F32)
    nc.scalar.mul(den_s[:], den[:], float(S))
    rden = const.tile([O, 1], F32)
    nc.vector.reciprocal(rden[:], den_s[:])
    attn_n = const.tile([O, 16], F32)
    nc.vector.tensor_scalar_mul(attn_n[:], att_e[:], rden[:, :])

    # Weff[o, c, k] = attn_n[o, c] * Wfc[o, c, k]   (k = 36 spatial), bf16
    weff = const.tile([O, CK], BF16)
    w_v = wsb.rearrange("p (c k) -> p c k", c=C)
    a_v = attn_n[:].rearrange("p (c k) -> p c k", k=1)
    o_v = weff[:].rearrange("p (c k) -> p c k", c=C)
    w_bc, a_bc = bass.broadcast_tensor_aps(w_v, a_v)
    nc.vector.tensor_tensor(o_v, w_bc, a_bc, op=OP.mult)

    # Weff^T chunks (bf16): wT[:, k*O:(k+1)*O] = Weff[:, chunk].T  ([kw, 83])
    wT = const.tile([128, 5 * O], BF16)
    for k, kw in enumerate(KC):
        c0 = 128 * k
        pt = ptr.tile([128, 128], BF16, tag="pt")
        nc.tensor.transpose(pt[0:kw, 0:O], weff[:, c0:c0 + kw], ident[0:O, 0:O])
        nc.scalar.copy(wT[0:kw, k * O:(k + 1) * O], pt[0:kw, 0:O])

    outsb = const.tile([O, BS], F32)

    # ---- main loop over batch tiles ------------------------------------
    for t in range(NT):
        b0 = t * P
        q0, q1, q2, q3, q4 = qs[t]

        # level A: pair adds within 4-slice chunks (2 pairs fused per op)
        def pairs(q):
            v = q[:].rearrange("p (a b k) -> p a b k", a=2, b=2)
            return v[:, :, 0, :], v[:, :, 1, :]

        a0 = apool.tile([P, 2 * CK], F32, tag="a0")
        s0a, s0b = pairs(q0)
        nc.vector.tensor_tensor(a0[:].rearrange("p (a k) -> p a k", a=2),
                                s0a, s0b, op=OP.add)
        a1 = apool.tile([P, 2 * CK], F32, tag="a1")
        s1a, s1b = pairs(q1)
        nc.vector.tensor_tensor(a1[:].rearrange("p (a k) -> p a k", a=2),
                                s1a, s1b, op=OP.add)
        a2 = apool.tile([P, 2 * CK], F32, tag="a2")
        s2a, s2b = pairs(q2)
        nc.gpsimd.tensor_tensor(a2[:].rearrange("p (a k) -> p a k", a=2),
                                s2a, s2b, op=OP.add)
        a3 = spool.tile([P, CK], F32, tag="a3")
        nc.gpsimd.tensor_add(a3[:], q3[:, 0:CK], q3[:, CK:2 * CK])

        # level B
        b0t = spool.tile([P, CK], F32, tag="b0")
        nc.vector.tensor_add(b0t[:], a0[:, 0:CK], a0[:, CK:2 * CK])
        b1t = spool.tile([P, CK], F32, tag="b1")
        nc.vector.tensor_add(b1t[:], a1[:, 0:CK], a1[:, CK:2 * CK])
        b2t = spool.tile([P, CK], F32, tag="b2")
        nc.gpsimd.tensor_add(b2t[:], a2[:, 0:CK], a2[:, CK:2 * CK])

        # level C + D; last add downcasts to bf16 for the PE stage
        c0t = spool.tile([P, CK], F32, tag="c0")
        nc.vector.tensor_add(c0t[:], b0t[:], b1t[:])
        c1t = spool.tile([P, CK], F32, tag="c1")
        nc.gpsimd.tensor_add(c1t[:], b2t[:], a3[:])
        ssa = spool.tile([P, CK], F32, tag="ssa")
        nc.vector.tensor_add(ssa[:], c0t[:], q4[:])
        ss = sspool.tile([P, CK], BF16, tag="ss")
        nc.vector.tensor_add(ss[:], ssa[:], c1t[:])

        # PE transpose the slice-sum: xT[:, k*P:(k+1)*P] = ss[:, chunk].T
        xT = xtp.tile([128, 5 * P], BF16)
        for k, kw in enumerate(KC):
            c0 = 128 * k
            pt = ptr.tile([128, 128], BF16, tag="pt")
            nc.tensor.transpose(pt[0:kw, :], ss[:, c0:c0 + kw], ident[:, :])
            nc.scalar.copy(xT[0:kw, k * P:(k + 1) * P], pt[0:kw, :])

        po = pout.tile([O, P], F32)
        for k, kw in enumerate(KC):
            nc.tensor.matmul(po[:], wT[0:kw, k * O:(k + 1) * O],
                             xT[0:kw, k * P:(k + 1) * P],
                             start=(k == 0), stop=(k == len(KC) - 1))

        nc.scalar.add(outsb[:, b0:b0 + P], po[:], bias)

    # ---- output store: 3 row-chunks on the 3 DMA-capable queues --------
    RS = [(0, 28), (28, 28), (56, 27)]
    engines = [nc.sync, nc.scalar, nc.gpsimd]
    for (r0, nr), eng in zip(RS, engines):
        eng.dma_start(o_d[r0:r0 + nr, :], outsb[r0:r0 + nr, :])


def build_program(repeat: int = 1):
    nc = bacc.Bacc("TRN2", target_bir_lowering=False, debug=False,
                   num_devices=N_CORES)
    x_d = nc.dram_tensor("x", [BS, SCK], F32, kind="ExternalInput").ap()
    s_d = nc.dram_tensor("setup", [O, 16 + CK + 1], F32,
                         kind="ExternalInput").ap()
    o_d = nc.dram_tensor("out", [O, BS], F32, kind="ExternalOutput").ap()

    with tile.TileContext(nc) as tc:
        if repeat == 1:
            with ExitStack() as ctx:
                _build_body(ctx, tc, o_d, x_d, s_d)
        else:
            def body(_iv):
                with ExitStack() as ctx:
                    _build_body(ctx, tc, o_d, x_d, s_d)
            tc.For_i_unrolled(0, repeat, 1, body, max_unroll=1)
    nc.compile()
    return nc


_NC_CACHE = {}


def _get_program(repeat: int = 1):
    if repeat not in _NC_CACHE:
        _NC_CACHE[repeat] = build_program(repeat)
    return _NC_CACHE[repeat]


class _Runner:
    """Cached jitted shard_map runner (mirrors bass2jax.run_bass_via_pjrt's
    multi-core path, but built once and fed full arrays without the per-core
    split + re-concat host copies)."""

    def __init__(self, nc):
        import jax
        from jax.sharding import Mesh, PartitionSpec, NamedSharding
        from jax.experimental.shard_map import shard_map
        from concourse import bass2jax
        from concourse.bass2jax import _bass_exec_p, install_neuronx_cc_hook

        install_neuronx_cc_hook()
        self.jax = jax
        pname = nc.partition_id_tensor.name if nc.partition_id_tensor else None
        in_names, out_names, out_avals, zeros = [], [], [], []
        for alloc in nc.m.functions[0].allocations:
            if not isinstance(alloc, mybir.MemoryLocationSet):
                continue
            name = alloc.memorylocations[0].name
            if alloc.kind == "ExternalInput":
                if name != pname:
                    in_names.append(name)
            elif alloc.kind == "ExternalOutput":
                shape = tuple(alloc.tensor_shape)
                dtype = mybir.dt.np(alloc.dtype)
                out_names.append(name)
                out_avals.append(jax.core.ShapedArray(shape, dtype))
                zeros.append(np.zeros((N_CORES * shape[0], *shape[1:]), dtype))
        self.in_names, self.out_names, self.zeros = in_names, out_names, zeros
        all_in = list(in_names) + list(out_names)
        if pname is not None:
            all_in.append(pname)

        def _body(*args):
            operands = list(args)
            if pname is not None:
                operands.append(bass2jax.partition_id_tensor())
            return tuple(_bass_exec_p.bind(
                *operands, out_avals=tuple(out_avals), in_names=tuple(all_in),
                out_names=tuple(out_names), lowering_input_output_aliases=(),
                sim_require_finite=True, sim_require_nnan=True, nc=nc))

        devices = jax.devices()[:N_CORES]
        mesh = Mesh(np.asarray(devices), ("core",))
        n_p, n_o = len(in_names), len(out_names)
        self.sharded = jax.jit(
            shard_map(_body, mesh=mesh,
                      in_specs=(PartitionSpec("core"),) * (n_p + n_o),
                      out_specs=(PartitionSpec("core"),) * n_o,
                      check_rep=False),
            donate_argnums=tuple(range(n_p, n_p + n_o)), keep_unused=True)
        self.sharding = NamedSharding(mesh, PartitionSpec("core"))

    def __call__(self, full_ins: dict):
        outs = self.sharded(*[full_ins[n] for n in self.in_names],
                            *[z.copy() for z in self.zeros])
        return {n: np.asarray(outs[i]) for i, n in enumerate(self.out_names)}


_RUNNER = None


def _pack_setup(inputs):
    """[83, 593]: cols 0:16 attn logits (heart;lung;lung), 16:592 fcw, 592 bias."""
    h = np.asarray(inputs["dzfeatweights_heart"], dtype=np.float32).reshape(NH, 16)
    l = np.asarray(inputs["dzfeatweights_lung"], dtype=np.float32).reshape(NL, 16)
    w = np.asarray(inputs["fclayers_weights"], dtype=np.float32).reshape(O, CK)
    b = np.asarray(inputs["fclayers_biases"], dtype=np.float32).reshape(O, 1)
    return np.concatenate([np.concatenate([h, l, l], axis=0), w, b],
                          axis=1).astype(np.float32)


def make_in_maps(inputs):
    x = np.asarray(inputs["x"], dtype=np.float32).reshape(B, SCK)
    s = _pack_setup(inputs)
    return [{"x": x[c * BS:(c + 1) * BS], "setup": s} for c in range(N_CORES)]


def assemble_output(results):
    outs = [results[c]["out"] for c in range(N_CORES)]    # each [83, 512]
    return np.ascontiguousarray(np.concatenate(outs, axis=1).T)  # [4096, 83]


def kernel(**inputs) -> np.ndarray:
    global _RUNNER
    if _RUNNER is None:
        _RUNNER = _Runner(_get_program(1))
    # Full (concatenated-over-cores) input arrays; x needs no copy at all.
    full = {
        "x": np.ascontiguousarray(
            np.asarray(inputs["x"], dtype=np.float32)).reshape(B, SCK),
        "setup": np.tile(_pack_setup(inputs), (N_CORES, 1)),
    }
    outs = _RUNNER(full)["out"]            # [8*83, 512]
    per_core = outs.reshape(N_CORES, O, BS)
    return np.ascontiguousarray(
        np.concatenate([per_core[c] for c in range(N_CORES)], axis=1).T)


# revision 8
# speedup vs baseline: 3.1716x; 1.1107x over previous
"""Trainium2 Bass kernel for nn_BodyAvgDiseaseFeatureAttn2.

Computation (reference):
    attn  = softmax over channels of [heart(27); lung(28); lung(28)] -> [83, 16]
    Weff[o,c,h,w] = attn[o,c] * Wfc[o,c,h,w]
    out[b,o] = mean_s( sum_{c,h,w} x[b,s,c,h,w] * Weff[o,c,h,w] ) + bias[o]

Kernel strategy (pure data parallel, 8 cores, batch-sharded), v2:
  - x per core is 17.7 MB; HBM stream at ~428 GB/s is the roofline
    (~41.5 us). All 4 batch tiles (138 KB/partition) fit in SBUF, so ALL
    x DMAs are issued up front (20 chunk DMAs, no buffer reuse -> no
    stalls), and compute overlaps the stream with large headroom.
  - slice-sum tree: fp32 pair-adds split DVE (chunks 0,1 + combines) /
    GPSIMD (chunk 2 + tail combines), fused via 3D APs (2 pairs/op).
  - final 576-dot on PE in BF16 (tree output downcast in its last add;
    Weff precomputed in bf16): transposes+matmuls ~2x faster, fp32 PSUM
    accumulation keeps error ~1e-3 << 2e-2 budget.
  - psum->sbuf copies and the bias add run on the Scalar engine.
  - output store split into 4 row-chunks on 4 queues (a single [83 x
    2 KB] store serializes ~95 ns/packet on one DMA engine: ~8 us).
"""

import numpy as np
from contextlib import ExitStack

import concourse.bass as bass
import concourse.bacc as bacc
import concourse.tile as tile
import concourse.mybir as mybir
from concourse import masks
from concourse.bass_utils import run_bass_kernel_spmd

F32 = mybir.dt.float32
BF16 = mybir.dt.bfloat16
AX = mybir.AxisListType
OP = mybir.AluOpType
ACT = mybir.ActivationFunctionType

N_CORES = 8
B, S, C, H, W = 4096, 15, 16, 6, 6
CK = C * H * W            # 576
SCK = S * CK              # 8640
NH, NL = 27, 28
O = 2 * NL + NH           # 83
BS = B // N_CORES         # 512 volumes per core
P = 128                   # partition tile
NT = BS // P              # 4 batch tiles per core
KC = [128, 128, 128, 128, 64]  # contraction chunking of 576
# slice chunking of each tile's DMA: big chunks keep the completion-
# semaphore rotation short (only ~8 sems exist; DMA k's issue waits for
# DMA k-8's completion). Tile 3 gets finer tail chunks.
QS3 = [(0, 8), (8, 4), (12, 2), (14, 1)]   # last tile
QSE = [(0, 8), (8, 7)]                     # tiles 0..2


def _build_body(ctx, tc, o_d, x_d, s_d):
    nc = tc.nc

    const = ctx.enter_context(tc.tile_pool(name="const", bufs=1))
    ptr = ctx.enter_context(tc.tile_pool(name="ptr", bufs=4, space="PSUM"))
    pout = ctx.enter_context(tc.tile_pool(name="pout", bufs=2, space="PSUM"))
    xpool = ctx.enter_context(tc.tile_pool(name="xp", bufs=1))
    apool = ctx.enter_context(tc.tile_pool(name="ap", bufs=1))
    spool = ctx.enter_context(tc.tile_pool(name="sp", bufs=1))
    sspool = ctx.enter_context(tc.tile_pool(name="ssp", bufs=2))
    xtp = ctx.enter_context(tc.tile_pool(name="xtp", bufs=2))

    # ---- setup DMA (packed [83, 593] tensor, 6 row-chunks), then x -----
    # All on the sync queue. A single [83 x 2.3 KB] DMA serializes its 83
    # packets on ONE DMA engine (~11 us); split into 6 row-chunks the
    # packets spread across engines (~1 us). Ditto the final store.
    setup = const.tile([O, 16 + CK + 1], F32)
    for r0 in range(0, O, 14):
        nr = min(14, O - r0)
        nc.sync.dma_start(setup[r0:r0 + nr, :], s_d[r0:r0 + nr, :])
    attn = setup[:, 0:16]
    wsb = setup[:, 16:16 + CK]
    bias = setup[:, 16 + CK:16 + CK + 1]

    qs = [[] for _ in range(NT)]
    for t in range(NT):
        b0 = t * P
        for (s0, ns) in (QS3 if t == NT - 1 else QSE):
            xq = xpool.tile([P, ns * CK], F32, tag=f"xq{t}_{s0}")
            nc.sync.dma_start(xq[:], x_d[b0:b0 + P, s0 * CK:(s0 + ns) * CK])
            qs[t].append(xq)

    # ---- constants / setup ---------------------------------------------
    ident = const.tile([128, 128], BF16)
    masks.make_identity(nc, ident[:])

    # softmax over the 16 channels, folded with the 1/S slice-average
    negmax = const.tile([O, 1], F32)
    nc.vector.tensor_reduce(negmax[:], attn, axis=AX.X, op=OP.max, negate=True)
    att_e = const.tile([O, 16], F32)
    den = const.tile([O, 1], F32)
    nc.scalar.activation(att_e[:], attn, ACT.Exp, bias=negmax[:, :], scale=1.0,
                         accum_out=den[:])
    den_s = const.tile([O, 1], F32)
    nc.scalar.mul(den_s[:], den[:], float(S))
    rden = const.tile([O, 1], F32)
    nc.vector.reciprocal(rden[:], den_s[:])
    attn_n = const.tile([O, 16], F32)
    nc.vector.tensor_scalar_mul(attn_n[:], att_e[:], rden[:, :])

    # Weff[o, c, k] = attn_n[o, c] * Wfc[o, c, k]   (k = 36 spatial), bf16
    weff = const.tile([O, CK], BF16)
    w_v = wsb.rearrange("p (c k) -> p c k", c=C)
    a_v = attn_n[:].rearrange("p (c k) -> p c k", k=1)
    o_v = weff[:].rearrange("p (c k) -> p c k", c=C)
    w_bc, a_bc = bass.broadcast_tensor_aps(w_v, a_v)
    nc.vector.tensor_tensor(o_v, w_bc, a_bc, op=OP.mult)

    # Weff^T chunks (bf16): wT[:, k*O:(k+1)*O] = Weff[:, chunk].T  ([kw, 83])
    wT = const.tile([128, 5 * O], BF16)
    for k, kw in enumerate(KC):
        c0 = 128 * k
        pt = ptr.tile([128, 128], BF16, tag="pt")
        nc.tensor.transpose(pt[0:kw, 0:O], weff[:, c0:c0 + kw], ident[0:O, 0:O])
        nc.scalar.copy(wT[0:kw, k * O:(k + 1) * O], pt[0:kw, 0:O])

    outsb = const.tile([O, BS], F32)

    # ---- main loop over batch tiles ------------------------------------
    # Slice-sum tree. DVE and GPSIMD run at roughly half speed while the
    # DMA stream is active, so level A downcasts to bf16 (half the SBUF
    # traffic; 16-bit DVE ops run 2x) and the rest of the tree is bf16.
    for t in range(NT):
        b0 = t * P
        if t == NT - 1:
            q0, q1, q2, q4 = qs[t]
            q2v = q2[:].rearrange("p (a k) -> p a k", a=2)
            p67a, p67b = q2v[:, 0, :], q2v[:, 1, :]       # s12, s13
            q4v = q4[:]                                    # s14
        else:
            q0, q1 = qs[t]
            q1v = q1[:].rearrange("p (a k) -> p a k", a=7)
            p67a, p67b = q1v[:, 4, :], q1v[:, 5, :]       # s12, s13
            q4v = q1v[:, 6, :]                             # s14

        # level A: pairs (0,1)..(6,7) from q0 on DVE; (8,9),(10,11) and
        # (12,13) on GPSIMD; all outputs bf16
        v0 = q0[:].rearrange("p (a b k) -> p a b k", a=4, b=2)
        a0 = apool.tile([P, 4 * CK], BF16, tag="a0")
        nc.vector.tensor_tensor(a0[:].rearrange("p (a k) -> p a k", a=4),
                                v0[:, :, 0, :], v0[:, :, 1, :], op=OP.add)
        if t == NT - 1:
            v1 = q1[:].rearrange("p (a b k) -> p a b k", a=2, b=2)
        else:
            v1 = q1[:, 0:4 * CK].rearrange("p (a b k) -> p a b k", a=2, b=2)
        a1 = apool.tile([P, 2 * CK], BF16, tag="a1")
        nc.gpsimd.tensor_tensor(a1[:].rearrange("p (a k) -> p a k", a=2),
                                v1[:, :, 0, :], v1[:, :, 1, :], op=OP.add)
        a2 = spool.tile([P, CK], BF16, tag="a2")
        nc.gpsimd.tensor_add(a2[:], p67a, p67b)

        # levels B..D (bf16)
        u0 = spool.tile([P, CK], BF16, tag="u0")
        nc.vector.tensor_add(u0[:], a0[:, 0:CK], a0[:, CK:2 * CK])
        u1 = spool.tile([P, CK], BF16, tag="u1")
        nc.vector.tensor_add(u1[:], a0[:, 2 * CK:3 * CK], a0[:, 3 * CK:4 * CK])
        u2 = spool.tile([P, CK], BF16, tag="u2")
        nc.gpsimd.tensor_add(u2[:], a1[:, 0:CK], a1[:, CK:2 * CK])
        w0 = spool.tile([P, CK], BF16, tag="w0")
        nc.vector.tensor_add(w0[:], u0[:], u1[:])
        w1 = spool.tile([P, CK], BF16, tag="w1")
        nc.gpsimd.tensor_add(w1[:], u2[:], a2[:])
        ssa = spool.tile([P, CK], BF16, tag="ssa")
        nc.vector.tensor_add(ssa[:], w0[:], q4v)
        ss = sspool.tile([P, CK], BF16, tag="ss")
        nc.vector.tensor_add(ss[:], ssa[:], w1[:])

        # PE transpose the slice-sum: xT[:, k*P:(k+1)*P] = ss[:, chunk].T
        xT = xtp.tile([128, 5 * P], BF16)
        for k, kw in enumerate(KC):
            c0 = 128 * k
            pt = ptr.tile([128, 128], BF16, tag="pt")
            nc.tensor.transpose(pt[0:kw, :], ss[:, c0:c0 + kw], ident[:, :])
            nc.scalar.copy(xT[0:kw, k * P:(k + 1) * P], pt[0:kw, :])

        po = pout.tile([O, P], F32)
        for k, kw in enumerate(KC):
            nc.tensor.matmul(po[:], wT[0:kw, k * O:(k + 1) * O],
                             xT[0:kw, k * P:(k + 1) * P],
                             start=(k == 0), stop=(k == len(KC) - 1))

        nc.scalar.add(outsb[:, b0:b0 + P], po[:], bias)

    # ---- output store: 3 row-chunks on the 3 DMA-capable queues --------
    RS = [(0, 28), (28, 28), (56, 27)]
    engines = [nc.sync, nc.scalar, nc.gpsimd]
    for (r0, nr), eng in zip(RS, engines):
        eng.dma_start(o_d[r0:r0 + nr, :], outsb[r0:r0 + nr, :])


def build_program(repeat: int = 1):
    nc = bacc.Bacc("TRN2", target_bir_lowering=False, debug=False,
                   num_devices=N_CORES)
    x_d = nc.dram_tensor("x", [BS, SCK], F32, kind="ExternalInput").ap()
    s_d = nc.dram_tensor("setup", [O, 16 + CK + 1], F32,
                         kind="ExternalInput").ap()
    o_d = nc.dram_tensor("out", [O, BS], F32, kind="ExternalOutput").ap()

    with tile.TileContext(nc) as tc:
        if repeat == 1:
            with ExitStack() as ctx:
                _build_body(ctx, tc, o_d, x_d, s_d)
        else:
            def body(_iv):
                with ExitStack() as ctx:
                    _build_body(ctx, tc, o_d, x_d, s_d)
            tc.For_i_unrolled(0, repeat, 1, body, max_unroll=1)
    nc.compile()
    return nc


_NC_CACHE = {}


def _get_program(repeat: int = 1):
    if repeat not in _NC_CACHE:
        _NC_CACHE[repeat] = build_program(repeat)
    return _NC_CACHE[repeat]


class _Runner:
    """Cached jitted shard_map runner (mirrors bass2jax.run_bass_via_pjrt's
    multi-core path, but built once and fed full arrays without the per-core
    split + re-concat host copies)."""

    def __init__(self, nc):
        import jax
        from jax.sharding import Mesh, PartitionSpec, NamedSharding
        from jax.experimental.shard_map import shard_map
        from concourse import bass2jax
        from concourse.bass2jax import _bass_exec_p, install_neuronx_cc_hook

        install_neuronx_cc_hook()
        self.jax = jax
        pname = nc.partition_id_tensor.name if nc.partition_id_tensor else None
        in_names, out_names, out_avals, zeros = [], [], [], []
        for alloc in nc.m.functions[0].allocations:
            if not isinstance(alloc, mybir.MemoryLocationSet):
                continue
            name = alloc.memorylocations[0].name
            if alloc.kind == "ExternalInput":
                if name != pname:
                    in_names.append(name)
            elif alloc.kind == "ExternalOutput":
                shape = tuple(alloc.tensor_shape)
                dtype = mybir.dt.np(alloc.dtype)
                out_names.append(name)
                out_avals.append(jax.core.ShapedArray(shape, dtype))
                zeros.append(np.zeros((N_CORES * shape[0], *shape[1:]), dtype))
        self.in_names, self.out_names, self.zeros = in_names, out_names, zeros
        all_in = list(in_names) + list(out_names)
        if pname is not None:
            all_in.append(pname)

        def _body(*args):
            operands = list(args)
            if pname is not None:
                operands.append(bass2jax.partition_id_tensor())
            return tuple(_bass_exec_p.bind(
                *operands, out_avals=tuple(out_avals), in_names=tuple(all_in),
                out_names=tuple(out_names), lowering_input_output_aliases=(),
                sim_require_finite=True, sim_require_nnan=True, nc=nc))

        devices = jax.devices()[:N_CORES]
        mesh = Mesh(np.asarray(devices), ("core",))
        n_p, n_o = len(in_names), len(out_names)
        self.sharded = jax.jit(
            shard_map(_body, mesh=mesh,
                      in_specs=(PartitionSpec("core"),) * (n_p + n_o),
                      out_specs=(PartitionSpec("core"),) * n_o,
                      check_rep=False),
            donate_argnums=tuple(range(n_p, n_p + n_o)), keep_unused=True)
        self.sharding = NamedSharding(mesh, PartitionSpec("core"))

    def __call__(self, full_ins: dict):
        outs = self.sharded(*[full_ins[n] for n in self.in_names],
                            *[z.copy() for z in self.zeros])
        return {n: np.asarray(outs[i]) for i, n in enumerate(self.out_names)}


_RUNNER = None


def _pack_setup(inputs):
    """[83, 593]: cols 0:16 attn logits (heart;lung;lung), 16:592 fcw, 592 bias."""
    h = np.asarray(inputs["dzfeatweights_heart"], dtype=np.float32).reshape(NH, 16)
    l = np.asarray(inputs["dzfeatweights_lung"], dtype=np.float32).reshape(NL, 16)
    w = np.asarray(inputs["fclayers_weights"], dtype=np.float32).reshape(O, CK)
    b = np.asarray(inputs["fclayers_biases"], dtype=np.float32).reshape(O, 1)
    return np.concatenate([np.concatenate([h, l, l], axis=0), w, b],
                          axis=1).astype(np.float32)


def make_in_maps(inputs):
    x = np.asarray(inputs["x"], dtype=np.float32).reshape(B, SCK)
    s = _pack_setup(inputs)
    return [{"x": x[c * BS:(c + 1) * BS], "setup": s} for c in range(N_CORES)]


def assemble_output(results):
    outs = [results[c]["out"] for c in range(N_CORES)]    # each [83, 512]
    return np.ascontiguousarray(np.concatenate(outs, axis=1).T)  # [4096, 83]


def kernel(**inputs) -> np.ndarray:
    global _RUNNER
    if _RUNNER is None:
        _RUNNER = _Runner(_get_program(1))
    # Full (concatenated-over-cores) input arrays; x needs no copy at all.
    full = {
        "x": np.ascontiguousarray(
            np.asarray(inputs["x"], dtype=np.float32)).reshape(B, SCK),
        "setup": np.tile(_pack_setup(inputs), (N_CORES, 1)),
    }
    outs = _RUNNER(full)["out"]            # [8*83, 512]
    per_core = outs.reshape(N_CORES, O, BS)
    return np.ascontiguousarray(
        np.concatenate([per_core[c] for c in range(N_CORES)], axis=1).T)
